# revision 11
# baseline (speedup 1.0000x reference)
"""Multi-head attention (B=2, S=2048, D=1024, H=16) on 8 Trainium2 cores.

Sharding: core c handles batch c//4 and head-group c%4 (4 heads x dk 64).
Q/K/V projection weights are column-split by head group on the host; the
output projection is split by OUTPUT column: core c computes all 2048
tokens x its 256 output columns, so each core consumes the full gathered
concat but no final collective or dynamic slice is needed.

Attention runs in 4 chunks of 512 query tokens x 4 heads.  Scores stay in
[k, q] orientation; the PV product streams exp-scores against a stationary
V slice plus a ones column, giving [dk+1, q] with the softmax denominator
in row dk.  Normalization: DVE fast-reciprocal on the denominator row ->
gpsimd partition-broadcast -> fused multiply.  Scores of block i+1
interleave with the PV matmuls of block i so PE and ACT stay busy.

As soon as a chunk's 4 heads are normalized, an AllGather ships the
[256, 512] per-head outputs inside each 4-core batch group; the output
projection for that chunk is interleaved into a later chunk's attention
stream so the collective latency is hidden.  Only the last chunk's
AllGather + projection are exposed (~25us tail).
"""

import numpy as np
import ml_dtypes

import concourse.bass as bass
import concourse.tile as tile
from concourse import bacc, mybir
from concourse.bass_utils import run_bass_kernel_spmd

BF16 = mybir.dt.bfloat16
F32 = mybir.dt.float32
NPBF16 = ml_dtypes.bfloat16

B, S, D, H = 2, 2048, 1024, 16
DK = 64
DK1 = DK + 1
N_CORES = 8
HPC = 4               # heads per core
FEAT = HPC * DK       # 256 projected features per core
VW = HPC * DK1        # 260: v with a ones column per head
OCOL = 256            # output columns per core
TOKC = 1024           # token chunk for projections
QCH = 512             # q chunk for attention (= AllGather granularity)
NCH = S // QCH        # 4 chunks
NKT = S // 128        # 16 k tiles
NKC = D // 128        # 8 contraction chunks

_CACHE = {}


def _build_program():
    if "nc" in _CACHE:
        return _CACHE["nc"]

    nc = bacc.Bacc("TRN2", target_bir_lowering=False, debug=False,
                   num_devices=N_CORES)

    xq = nc.declare_dram_parameter("xq", [D, S], BF16, isOutput=False)
    xk = nc.declare_dram_parameter("xk", [D, S], BF16, isOutput=False)
    xv = nc.declare_dram_parameter("xv", [D, S], BF16, isOutput=False)
    wq = nc.declare_dram_parameter("wq", [D, FEAT], BF16, isOutput=False)
    wk = nc.declare_dram_parameter("wk", [D, FEAT], BF16, isOutput=False)
    wv = nc.declare_dram_parameter("wv", [D, VW], BF16, isOutput=False)
    wo = nc.declare_dram_parameter("wo", [D, OCOL], BF16, isOutput=False)
    bq = nc.declare_dram_parameter("bq", [128, 2], F32, isOutput=False)
    bk = nc.declare_dram_parameter("bk", [128, 2], F32, isOutput=False)
    bv = nc.declare_dram_parameter("bv", [1, VW], BF16, isOutput=False)
    bo = nc.declare_dram_parameter("bo", [1, OCOL], BF16, isOutput=False)
    out = nc.declare_dram_parameter("out", [S, OCOL], BF16, isOutput=True)
    dbg = {}
    if _CACHE.get("debug"):
        dbg["kh"] = nc.declare_dram_parameter("dbg_kh", [128, S], BF16,
                                              isOutput=True)
        dbg["qh"] = nc.declare_dram_parameter("dbg_qh", [128, S], BF16,
                                              isOutput=True)
        dbg["v0"] = nc.declare_dram_parameter("dbg_v0", [128, VW], BF16,
                                              isOutput=True)
        dbg["sct"] = nc.declare_dram_parameter("dbg_sct", [128, 2 * QCH],
                                               BF16, isOutput=True)
        dbg["pvs"] = nc.declare_dram_parameter("dbg_pvs", [DK1, QCH], F32,
                                               isOutput=True)
        dbg["cat"] = nc.declare_dram_parameter("dbg_cat", [128, QCH], BF16,
                                               isOutput=True)
        dbg["onrm"] = nc.declare_dram_parameter("dbg_onrm", [DK, QCH], BF16,
                                                isOutput=True)
        dbg["agin"] = nc.declare_dram_parameter("dbg_agin", [FEAT, QCH],
                                                BF16, isOutput=True)
        dbg["agor"] = nc.declare_dram_parameter("dbg_agor", [FEAT, QCH],
                                                BF16, isOutput=True)

    with tile.TileContext(nc) as tc:
        with (
            tc.tile_pool(name="w", bufs=1) as wpool,
            tc.tile_pool(name="x", bufs=26) as xpool,
            tc.tile_pool(name="qk", bufs=1) as qkpool,
            tc.tile_pool(name="vp", bufs=1) as vpool,
            tc.tile_pool(name="sct", bufs=16) as sctpool,
            tc.tile_pool(name="nm", bufs=2) as nmpool,
            tc.tile_pool(name="cat", bufs=16) as catpool,
            tc.tile_pool(name="fo", bufs=3) as fopool,
            tc.tile_pool(name="ps_a", bufs=2, space="PSUM") as ps_a,
            tc.tile_pool(name="ps_pv", bufs=2, space="PSUM") as ps_pv,
            tc.tile_pool(name="ps_o", bufs=2, space="PSUM") as ps_o,
            tc.tile_pool(name="dram", bufs=1, space="DRAM") as dram,
        ):
            _emit(nc, wpool, xpool, qkpool, vpool, sctpool, nmpool,
                  catpool, fopool, ps_a, ps_pv, ps_o, dram,
                  xq, xk, xv, wq, wk, wv, wo, bq, bk, bv, bo, out, dbg)

    nc.compile()
    _CACHE["nc"] = nc
    return nc


def _emit(nc, wpool, xpool, qkpool, vpool, sctpool, nmpool, catpool,
          fopool, ps_a, ps_pv, ps_o, dram,
          xq, xk, xv, wq, wk, wv, wo, bq, bk, bv, bo, out, dbg={}):
    MUL = mybir.AluOpType.mult
    EXPF = mybir.ActivationFunctionType.Exp
    IDF = mybir.ActivationFunctionType.Identity

    ones1 = wpool.tile([1, 128], BF16, tag="ones")
    nc.vector.memset(ones1[:], 1.0)

    # DMA issue engines for bulk input loads (round-robin: the Sync engine
    # alone issues descriptors at ~600ns each, which gates phase 1).  Only
    # SP and Activation are hardware-DGE engines; gpsimd DMA goes through
    # the software-DGE ring and corrupts data in this flow.
    dmae = [nc.sync, nc.scalar]
    NE = len(dmae)

    wk_sb = []
    for kc in range(NKC):
        t = wpool.tile([128, FEAT], BF16, tag=f"wk{kc}")
        dmae[kc % NE].dma_start(t[:], wk[bass.ts(kc, 128), :])
        wk_sb.append(t)
    bk_sb = wpool.tile([128, 2], F32, tag="bk")
    nc.sync.dma_start(bk_sb[:], bk[:])

    qh_sb = [qkpool.tile([128, S], BF16, tag=f"qh{m}", name=f"qh{m}")
             for m in range(2)]
    kh_sb = [qkpool.tile([128, S], BF16, tag=f"kh{m}", name=f"kh{m}")
             for m in range(2)]
    v_sb = [vpool.tile([128, VW], BF16, tag=f"v{j}", name=f"v{j}")
            for j in range(NKT)]

    def load_x(src, t0):
        tiles = []
        for kc in range(NKC):
            t = xpool.tile([128, TOKC], BF16, tag="xt")
            dmae[kc % NE].dma_start(t[:], src[bass.ts(kc, 128),
                                              bass.ts(t0, TOKC)])
            tiles.append(t)
        return tiles

    def qk_group(w_sb, x_t, b_sb, dst, t0, m):
        ps = ps_a.tile([128, TOKC], F32, tag="a")
        for kc in range(NKC):
            for u in range(TOKC // 512):
                nc.tensor.matmul(
                    ps[:, bass.ts(u, 512)],
                    w_sb[kc][:, bass.ts(m, 128)],
                    x_t[kc][:, bass.ts(u, 512)],
                    start=(kc == 0), stop=(kc == NKC - 1),
                )
        nc.vector.tensor_scalar_add(dst[m][:, bass.ts(t0, TOKC)], ps[:],
                                    b_sb[:, m:m + 1])

    # ---- K projection (scores need the full kh) ------------------
    xk_ts = [load_x(xk, t0) for t0 in range(S // TOKC)]
    for t0 in range(S // TOKC):
        for m in range(2):
            qk_group(wk_sb, xk_ts[t0], bk_sb, kh_sb, t0, m)

    wq_sb = []
    for kc in range(NKC):
        t = wpool.tile([128, FEAT], BF16, tag=f"wq{kc}")
        dmae[kc % NE].dma_start(t[:], wq[bass.ts(kc, 128), :])
        wq_sb.append(t)
    bq_sb = wpool.tile([128, 2], F32, tag="bq")
    nc.sync.dma_start(bq_sb[:], bq[:])
    for t0 in range(S // TOKC):
        xq_t = load_x(xq, t0)
        for m in range(2):
            qk_group(wq_sb, xq_t, bq_sb, qh_sb, t0, m)

    # ---- V weights + inputs --------------------------------------
    wv_sb = []
    for kc in range(NKC):
        t = wpool.tile([128, VW], BF16, tag=f"wv{kc}")
        dmae[kc % NE].dma_start(t[:], wv[bass.ts(kc, 128), :])
        wv_sb.append(t)
    bv_sb = wpool.tile([1, VW], BF16, tag="bv")
    nc.sync.dma_start(bv_sb[:], bv[:])
    xv_ts = [load_x(xv, t0) for t0 in range(S // TOKC)]

    def v_group(t0, j):
        ps = ps_a.tile([128, VW], F32, tag="a")
        for kc in range(NKC):
            nc.tensor.matmul(
                ps[:], xv_ts[t0][kc][:, bass.ts(j, 128)], wv_sb[kc][:],
                start=(kc == 0), stop=False,
            )
        nc.tensor.matmul(ps[:], ones1[:], bv_sb[:], start=False, stop=True)
        nc.vector.tensor_copy(v_sb[t0 * (TOKC // 128) + j][:], ps[:])

    for t0 in range(S // TOKC):
        for j in range(TOKC // 128):
            v_group(t0, j)

    # wo + bo requested now: the 0.5 MB load drains during attention.
    wo_sb = []
    for kc in range(NKC):
        t = wpool.tile([128, OCOL], BF16, tag=f"wo{kc}")
        dmae[kc % NE].dma_start(t[:], wo[bass.ts(kc, 128), :])
        wo_sb.append(t)
    bo_sb = wpool.tile([1, OCOL], BF16, tag="bo")
    nc.sync.dma_start(bo_sb[:], bo[:])

    # ---- phase 2/3: attention + chunked AllGather + out proj -----
    ag_in = [dram.tile([FEAT, QCH], BF16, tag=f"agi{c}", name=f"agi{c}")
             for c in range(NCH)]
    # chunks 0-2 gather in one piece; chunk 3 in two half-gathers (2 heads
    # each) so most of the final collective overlaps the last blocks.
    ag_out = [dram.tile([4 * FEAT, QCH], BF16, tag=f"ago{c}", name=f"ago{c}")
              for c in range(NCH - 1)]
    ag_out3 = [dram.tile([4 * 2 * DK, QCH], BF16, tag=f"ago3{p}",
                         name=f"ago3{p}") for p in range(2)]

    def emit_ag(ins_ap, outs_ap):
        nc.gpsimd.collective_compute(
            "AllGather", mybir.AluOpType.bypass,
            replica_groups=[[0, 1, 2, 3], [4, 5, 6, 7]],
            ins=[ins_ap.opt()],
            outs=[outs_ap.opt()],
        )

    def norm_and_out(pv, h, c):
        pvs = nmpool.tile([DK1, QCH], F32, tag="pvs")
        nc.vector.tensor_copy(pvs[:], pv[:])
        if dbg and (c, h) == (0, 0):
            nc.sync.dma_start(dbg["pvs"][:], pvs[:])
        drow = nmpool.tile([1, QCH], F32, tag="drow")
        nc.vector.tensor_copy(drow[:], pvs[DK:DK1, :])
        db = nmpool.tile([DK, QCH], F32, tag="db")
        nc.gpsimd.partition_broadcast(db[:], drow[:])
        rb = nmpool.tile([DK, QCH], F32, tag="rb")
        nc.vector.reciprocal_approx_fast(rb[:], db[:])
        onrm = nmpool.tile([DK, QCH], BF16, tag="onrm")
        nc.vector.scalar_tensor_tensor(onrm[:], pvs[0:DK, :], 1.0, rb[:],
                                       MUL, MUL)
        if dbg and (c, h) == (0, 0):
            nc.sync.dma_start(dbg["onrm"][:], onrm[:])
        nc.sync.dma_start(ag_in[c][h * DK:(h + 1) * DK, :], onrm[:])
        if c < NCH - 1:
            if h == HPC - 1:
                emit_ag(ag_in[c][:], ag_out[c][:])
        elif h == 1 or h == HPC - 1:
            p = h // 2
            emit_ag(ag_in[c][p * 2 * DK:(p + 1) * 2 * DK, :], ag_out3[p][:])

    def out_proj(c):
        cat = []
        for kc in range(NKC):
            t = catpool.tile([128, QCH], BF16, tag="cat")
            if c < NCH - 1:
                src = ag_out[c][bass.ts(kc, 128), :]
            else:
                # features kc*128 = core kc//2, head-pair kc%2
                src = ag_out3[kc % 2][bass.ts(kc // 2, 128), :]
            nc.sync.dma_start(t[:], src)
            if dbg and c == 0 and kc == 0:
                nc.sync.dma_start(dbg["cat"][:], t[:])
            cat.append(t)
        if dbg and c == 0:
            for kc in range(2):
                tb = catpool.tile([128, QCH], BF16, tag="cat")
                nc.sync.dma_start(tb[:], ag_in[0][bass.ts(kc, 128), :])
                nc.sync.dma_start(dbg["agin"][bass.ts(kc, 128), :], tb[:])
            for kc in range(2):
                tb = catpool.tile([128, QCH], BF16, tag="cat")
                nc.sync.dma_start(tb[:], ag_out[0][bass.ts(kc + 2, 128), :])
                nc.sync.dma_start(dbg["agor"][bass.ts(kc, 128), :], tb[:])
        fos = []
        for qt in range(QCH // 128):
            ps = ps_o.tile([128, OCOL], F32, tag="o")
            nc.tensor.matmul(ps[:], ones1[:, 0:128], bo_sb[:],
                             start=True, stop=False)
            for kc in range(NKC):
                nc.tensor.matmul(
                    ps[:],
                    cat[kc][:, bass.ts(qt, 128)],
                    wo_sb[kc][:],
                    start=False, stop=(kc == NKC - 1),
                )
            fo = fopool.tile([128, OCOL], BF16, tag="fo")
            nc.vector.tensor_copy(fo[:], ps[:])
            fos.append(fo)
        for qt, fo in enumerate(fos):
            nc.sync.dma_start(out[bass.ts(c * (QCH // 128) + qt, 128), :],
                              fo[:])

    if dbg:
        nc.sync.dma_start(dbg["kh"][:], kh_sb[0][:])
        nc.sync.dma_start(dbg["qh"][:], qh_sb[0][:])
        nc.sync.dma_start(dbg["v0"][:], v_sb[0][:])

    blocks = [(c, h) for c in range(NCH) for h in range(HPC)]
    prev = None
    for bi, (c, h) in enumerate(blocks):
        ht, hr = h // 2, (h % 2) * 64
        q0 = c * QCH
        pv = ps_pv.tile([DK1, QCH], F32, tag="pv")
        cur_sc = []
        for kp in range(NKT // 2):
            ps = ps_a.tile([128, 2 * QCH], F32, tag="a")
            sct = sctpool.tile([128, 2 * QCH], BF16, tag="sct", name="sct")
            for half in range(2):
                kt = 2 * kp + half
                nc.tensor.matmul(
                    ps[:, bass.ts(half, QCH)],
                    kh_sb[ht][hr:hr + 64, bass.ts(kt, 128)],
                    qh_sb[ht][hr:hr + 64, q0:q0 + QCH],
                    start=True, stop=True,
                )
                if prev is not None:
                    ppv, psc, ph, pc = prev
                    nc.tensor.matmul(
                        ppv[:],
                        v_sb[kt][:, ph * DK1:(ph + 1) * DK1],
                        psc[kp][:, bass.ts(half, QCH)],
                        start=(kt == 0), stop=(kt == NKT - 1),
                    )
            nc.scalar.activation(sct[:], ps[:], EXPF, scale=0.125)
            if dbg and bi == 0 and kp == 0:
                nc.sync.dma_start(dbg["sct"][:], sct[:])
            cur_sc.append(sct)
        if prev is not None:
            ppv, psc, ph, pc = prev
            norm_and_out(ppv, ph, pc)
        prev = (pv, cur_sc, h, c)
        # Interleave finished chunks' output projections into the attention
        # stream with >= 40us of lead over the AllGather that feeds them.
        if (c, h) == (2, 2):
            out_proj(0)
        elif (c, h) == (3, 1):
            out_proj(1)
        elif (c, h) == (3, 3):
            out_proj(2)

    # drain: the last block's PV ran nowhere, so run it standalone.
    ppv, psc, ph, pc = prev
    for kt in range(NKT):
        nc.tensor.matmul(
            ppv[:],
            v_sb[kt][:, ph * DK1:(ph + 1) * DK1],
            psc[kt // 2][:, bass.ts(kt % 2, QCH)],
            start=(kt == 0), stop=(kt == NKT - 1),
        )
    norm_and_out(ppv, ph, pc)
    out_proj(3)


def _prep_inputs(q, k, v, Wq, bq, Wk, bk, Wv, bv, Wo, bo):
    """Build the per-core input maps (host-side sharding)."""
    in_maps = []
    for c in range(N_CORES):
        b, hg = c // 4, c % 4
        fsl = slice(FEAT * hg, FEAT * (hg + 1))
        osl = slice(OCOL * hg, OCOL * (hg + 1))
        wv_aug = np.zeros((D, VW), np.float32)
        bv_aug = np.zeros((VW,), np.float32)
        for h in range(HPC):
            rows = slice(FEAT * hg + DK * h, FEAT * hg + DK * (h + 1))
            wv_aug[:, h * DK1:h * DK1 + DK] = Wv[rows, :].T
            bv_aug[h * DK1:h * DK1 + DK] = bv[rows]
            bv_aug[h * DK1 + DK] = 1.0
        in_maps.append({
            "xq": np.ascontiguousarray(q[b].T).astype(NPBF16),
            "xk": np.ascontiguousarray(k[b].T).astype(NPBF16),
            "xv": np.ascontiguousarray(v[b].T).astype(NPBF16),
            "wq": np.ascontiguousarray(Wq[fsl].T).astype(NPBF16),
            "wk": np.ascontiguousarray(Wk[fsl].T).astype(NPBF16),
            "wv": wv_aug.astype(NPBF16),
            "wo": np.ascontiguousarray(Wo[osl].T).astype(NPBF16),
            "bq": np.ascontiguousarray(
                bq[fsl].reshape(2, 128).T).astype(np.float32),
            "bk": np.ascontiguousarray(
                bk[fsl].reshape(2, 128).T).astype(np.float32),
            "bv": bv_aug.reshape(1, VW).astype(NPBF16),
            "bo": np.ascontiguousarray(
                bo[osl].reshape(1, OCOL)).astype(NPBF16),
        })
    return in_maps


def run_sharded(in_maps, trace=False):
    nc = _build_program()
    res = run_bass_kernel_spmd(nc, in_maps, list(range(N_CORES)), trace=trace)
    full = np.empty((B, S, D), np.float32)
    for c in range(N_CORES):
        b, hg = c // 4, c % 4
        full[b, :, OCOL * hg:OCOL * (hg + 1)] = (
            res.results[c]["out"].astype(np.float32))
    return full, res


def kernel(q, k, v, Wq, bq, Wk, bk, Wv, bv, Wo, bo):
    args = [np.asarray(x, np.float32) for x in
            (q, k, v, Wq, bq, Wk, bk, Wv, bv, Wo, bo)]
    in_maps = _prep_inputs(*args)
    full, _ = run_sharded(in_maps)
    return full


# revision 12
# speedup vs baseline: 1.0781x; 1.0781x over previous
"""Multi-head attention (B=2, S=2048, D=1024, H=16) on 8 Trainium2 cores.

Sharding: core c handles batch c//4 and head-group c%4 (4 heads x dk 64).
Q/K/V projection weights are column-split by head group on the host; the
output projection is split by OUTPUT column: core c computes all 2048
tokens x its 256 output columns, so each core consumes the full gathered
concat but no final collective or dynamic slice is needed.

Attention runs in 4 chunks of 512 query tokens x 4 heads.  Scores stay in
[k, q] orientation; the PV product streams exp-scores against a stationary
V slice plus a ones column, giving [dk+1, q] with the softmax denominator
in row dk.  Normalization: DVE fast-reciprocal on the denominator row ->
gpsimd partition-broadcast -> fused multiply.  Scores of block i+1
interleave with the PV matmuls of block i so PE and ACT stay busy.

As soon as a chunk's 4 heads are normalized, an AllGather ships the
[256, 512] per-head outputs inside each 4-core batch group; the output
projection for that chunk is interleaved into a later chunk's attention
stream so the collective latency is hidden.  Only the last chunk's
AllGather + projection are exposed (~25us tail).
"""

import numpy as np
import ml_dtypes

import concourse.bass as bass
import concourse.tile as tile
from concourse import bacc, mybir
from concourse.bass_utils import run_bass_kernel_spmd

BF16 = mybir.dt.bfloat16
F32 = mybir.dt.float32
NPBF16 = ml_dtypes.bfloat16

B, S, D, H = 2, 2048, 1024, 16
DK = 64
DK1 = DK + 1
N_CORES = 8
HPC = 4               # heads per core
FEAT = HPC * DK       # 256 projected features per core
VW = HPC * DK1        # 260: v with a ones column per head
OCOL = 256            # output columns per core
TOKC = 1024           # token chunk for projections
QCH = 512             # q chunk for attention (= AllGather granularity)
NCH = S // QCH        # 4 chunks
NKT = S // 128        # 16 k tiles
NKC = D // 128        # 8 contraction chunks

_CACHE = {}


def _build_program():
    if "nc" in _CACHE:
        return _CACHE["nc"]

    nc = bacc.Bacc("TRN2", target_bir_lowering=False, debug=False,
                   num_devices=N_CORES)

    xq = nc.declare_dram_parameter("xq", [D, S], BF16, isOutput=False)
    xk = nc.declare_dram_parameter("xk", [D, S], BF16, isOutput=False)
    xv = nc.declare_dram_parameter("xv", [D, S], BF16, isOutput=False)
    wq = nc.declare_dram_parameter("wq", [D, FEAT], BF16, isOutput=False)
    wk = nc.declare_dram_parameter("wk", [D, FEAT], BF16, isOutput=False)
    wv = nc.declare_dram_parameter("wv", [D, VW], BF16, isOutput=False)
    wo = nc.declare_dram_parameter("wo", [D, OCOL], BF16, isOutput=False)
    bq = nc.declare_dram_parameter("bq", [128, 2], F32, isOutput=False)
    bk = nc.declare_dram_parameter("bk", [128, 2], F32, isOutput=False)
    bv = nc.declare_dram_parameter("bv", [1, VW], BF16, isOutput=False)
    bo = nc.declare_dram_parameter("bo", [1, OCOL], BF16, isOutput=False)
    out = nc.declare_dram_parameter("out", [S, OCOL], BF16, isOutput=True)
    dbg = {}

    with tile.TileContext(nc) as tc:
        with (
            tc.tile_pool(name="w", bufs=1) as wpool,
            tc.tile_pool(name="x", bufs=26) as xpool,
            tc.tile_pool(name="qk", bufs=1) as qkpool,
            tc.tile_pool(name="vp", bufs=1) as vpool,
            tc.tile_pool(name="sct", bufs=18) as sctpool,
            tc.tile_pool(name="nm", bufs=2) as nmpool,
            tc.tile_pool(name="cat", bufs=16) as catpool,
            tc.tile_pool(name="fo", bufs=3) as fopool,
            tc.tile_pool(name="ps_a", bufs=3, space="PSUM") as ps_a,
            tc.tile_pool(name="ps_pv", bufs=1, space="PSUM") as ps_pv,
            tc.tile_pool(name="dram", bufs=1, space="DRAM") as dram,
        ):
            _emit(nc, wpool, xpool, qkpool, vpool, sctpool, nmpool,
                  catpool, fopool, ps_a, ps_pv, dram,
                  xq, xk, xv, wq, wk, wv, wo, bq, bk, bv, bo, out, dbg)

    nc.compile()
    _CACHE["nc"] = nc
    return nc


def _emit(nc, wpool, xpool, qkpool, vpool, sctpool, nmpool, catpool,
          fopool, ps_a, ps_pv, dram,
          xq, xk, xv, wq, wk, wv, wo, bq, bk, bv, bo, out, dbg={}):
    MUL = mybir.AluOpType.mult
    EXPF = mybir.ActivationFunctionType.Exp
    IDF = mybir.ActivationFunctionType.Identity

    ones1 = wpool.tile([1, 128], BF16, tag="ones")
    nc.vector.memset(ones1[:], 1.0)

    # DMA issue engines for bulk input loads (round-robin: the Sync engine
    # alone issues descriptors at ~600ns each, which gates phase 1).  Only
    # SP and Activation are hardware-DGE engines; gpsimd DMA goes through
    # the software-DGE ring and corrupts data in this flow.
    dmae = [nc.sync, nc.scalar]
    NE = len(dmae)

    wk_sb = []
    for kc in range(NKC):
        t = wpool.tile([128, FEAT], BF16, tag=f"wk{kc}")
        dmae[kc % NE].dma_start(t[:], wk[bass.ts(kc, 128), :])
        wk_sb.append(t)
    bk_sb = wpool.tile([128, 2], F32, tag="bk")
    nc.sync.dma_start(bk_sb[:], bk[:])

    qh_sb = [qkpool.tile([128, S], BF16, tag=f"qh{m}", name=f"qh{m}")
             for m in range(2)]
    kh_sb = [qkpool.tile([128, S], BF16, tag=f"kh{m}", name=f"kh{m}")
             for m in range(2)]
    v_sb = [vpool.tile([128, VW], BF16, tag=f"v{j}", name=f"v{j}")
            for j in range(NKT)]

    def load_x(src, t0):
        tiles = []
        for kc in range(NKC):
            t = xpool.tile([128, TOKC], BF16, tag="xt")
            dmae[kc % NE].dma_start(t[:], src[bass.ts(kc, 128),
                                              bass.ts(t0, TOKC)])
            tiles.append(t)
        return tiles

    def qk_group(w_sb, x_t, b_sb, dst, t0, m):
        ps = ps_a.tile([128, TOKC], F32, tag="a")
        for kc in range(NKC):
            for u in range(TOKC // 512):
                nc.tensor.matmul(
                    ps[:, bass.ts(u, 512)],
                    w_sb[kc][:, bass.ts(m, 128)],
                    x_t[kc][:, bass.ts(u, 512)],
                    start=(kc == 0), stop=(kc == NKC - 1),
                )
        nc.vector.tensor_scalar_add(dst[m][:, bass.ts(t0, TOKC)], ps[:],
                                    b_sb[:, m:m + 1])

    # ---- K projection (scores need the full kh) ------------------
    xk_ts = [load_x(xk, t0) for t0 in range(S // TOKC)]
    for t0 in range(S // TOKC):
        for m in range(2):
            qk_group(wk_sb, xk_ts[t0], bk_sb, kh_sb, t0, m)

    wq_sb = []
    for kc in range(NKC):
        t = wpool.tile([128, FEAT], BF16, tag=f"wq{kc}")
        dmae[kc % NE].dma_start(t[:], wq[bass.ts(kc, 128), :])
        wq_sb.append(t)
    bq_sb = wpool.tile([128, 2], F32, tag="bq")
    nc.sync.dma_start(bq_sb[:], bq[:])
    for t0 in range(S // TOKC):
        xq_t = load_x(xq, t0)
        for m in range(2):
            qk_group(wq_sb, xq_t, bq_sb, qh_sb, t0, m)

    # ---- V weights + inputs --------------------------------------
    wv_sb = []
    for kc in range(NKC):
        t = wpool.tile([128, VW], BF16, tag=f"wv{kc}")
        dmae[kc % NE].dma_start(t[:], wv[bass.ts(kc, 128), :])
        wv_sb.append(t)
    bv_sb = wpool.tile([1, VW], BF16, tag="bv")
    nc.sync.dma_start(bv_sb[:], bv[:])
    xv_ts = [load_x(xv, t0) for t0 in range(S // TOKC)]

    def v_group(t0, j):
        ps = ps_a.tile([128, VW], F32, tag="a")
        for kc in range(NKC):
            nc.tensor.matmul(
                ps[:], xv_ts[t0][kc][:, bass.ts(j, 128)], wv_sb[kc][:],
                start=(kc == 0), stop=False,
            )
        nc.tensor.matmul(ps[:], ones1[:], bv_sb[:], start=False, stop=True)
        nc.vector.tensor_copy(v_sb[t0 * (TOKC // 128) + j][:], ps[:])

    for t0 in range(S // TOKC):
        for j in range(TOKC // 128):
            v_group(t0, j)

    # wo + bo requested now: the 0.5 MB load drains during attention.
    wo_sb = []
    for kc in range(NKC):
        t = wpool.tile([128, OCOL], BF16, tag=f"wo{kc}")
        dmae[kc % NE].dma_start(t[:], wo[bass.ts(kc, 128), :])
        wo_sb.append(t)
    bo_sb = wpool.tile([1, OCOL], BF16, tag="bo")
    nc.sync.dma_start(bo_sb[:], bo[:])

    # ---- phase 2/3: attention + chunked AllGather + out proj -----
    # Attention runs in 2 q-blocks of 1024 x 4 heads (the baseline shape,
    # which paces best under the power throttle).  AllGathers fire per
    # 512-token chunk as soon as its heads are normalized; chunk 3 ships
    # in two half-gathers (heads 01 early, heads 23 at the end).  The
    # column-split output projections all run after attention, where the
    # ACT engine is quiet and the PE runs unthrottled; their collectives
    # are complete by then, so there is no dead zone.
    QB = 2 * QCH          # 1024-token attention block
    NQB = S // QB         # 2 blocks
    ag_in = [dram.tile([FEAT, QCH], BF16, tag=f"agi{c}", name=f"agi{c}")
             for c in range(NCH)]
    ag_out = [dram.tile([4 * FEAT, QCH], BF16, tag=f"ago{c}", name=f"ago{c}")
              for c in range(NCH - 1)]
    ag_out3 = [dram.tile([4 * 2 * DK, QCH], BF16, tag=f"ago3{p}",
                         name=f"ago3{p}") for p in range(2)]

    def emit_ag(ins_ap, outs_ap):
        nc.gpsimd.collective_compute(
            "AllGather", mybir.AluOpType.bypass,
            replica_groups=[[0, 1, 2, 3], [4, 5, 6, 7]],
            ins=[ins_ap.opt()],
            outs=[outs_ap.opt()],
        )

    def norm_and_out(pv, h, qb):
        pvs = nmpool.tile([DK1, QB], F32, tag="pvs")
        nc.vector.tensor_copy(pvs[:], pv[:])
        drow = nmpool.tile([1, QB], F32, tag="drow")
        nc.vector.tensor_copy(drow[:], pvs[DK:DK1, :])
        db = nmpool.tile([DK, QB], F32, tag="db")
        nc.gpsimd.partition_broadcast(db[:], drow[:])
        rb = nmpool.tile([DK, QB], F32, tag="rb")
        nc.vector.reciprocal_approx_fast(rb[:], db[:])
        onrm = nmpool.tile([DK, QB], BF16, tag="onrm")
        nc.vector.scalar_tensor_tensor(onrm[:], pvs[0:DK, :], 1.0, rb[:],
                                       MUL, MUL)
        for u in range(QB // QCH):
            c = qb * (QB // QCH) + u
            nc.sync.dma_start(ag_in[c][h * DK:(h + 1) * DK, :],
                              onrm[:, bass.ts(u, QCH)])
        if qb == NQB - 1 and h == 1:
            # chunk 3, heads 0-1 are complete: ship the first half early.
            emit_ag(ag_in[NCH - 1][0:2 * DK, :], ag_out3[0][:])
        if h == HPC - 1:
            for u in range(QB // QCH):
                c = qb * (QB // QCH) + u
                if c < NCH - 1:
                    emit_ag(ag_in[c][:], ag_out[c][:])
                else:
                    emit_ag(ag_in[c][2 * DK:4 * DK, :], ag_out3[1][:])

    def out_proj(c):
        cat = []
        for kc in range(NKC):
            t = catpool.tile([128, QCH], BF16, tag="cat")
            if c < NCH - 1:
                src = ag_out[c][bass.ts(kc, 128), :]
            else:
                # features kc*128 = core kc//2, head-pair kc%2
                src = ag_out3[kc % 2][bass.ts(kc // 2, 128), :]
            dmae[kc % NE].dma_start(t[:], src)
            cat.append(t)
        for qt in range(QCH // 128):
            ps = ps_a.tile([128, OCOL], F32, tag="a", name="po")
            nc.tensor.matmul(ps[:], ones1[:, 0:128], bo_sb[:],
                             start=True, stop=False)
            for kc in range(NKC):
                nc.tensor.matmul(
                    ps[:],
                    cat[kc][:, bass.ts(qt, 128)],
                    wo_sb[kc][:],
                    start=False, stop=(kc == NKC - 1),
                )
            fo = fopool.tile([128, OCOL], BF16, tag="fo")
            nc.scalar.activation(fo[:], ps[:], IDF)
            dmae[qt % NE].dma_start(
                out[bass.ts(c * (QCH // 128) + qt, 128), :], fo[:])

    blocks = [(qb, h) for qb in range(NQB) for h in range(HPC)]
    last = len(blocks) - 1
    prev = None
    for bi, (qb, h) in enumerate(blocks):
        ht, hr = h // 2, (h % 2) * 64
        q0 = qb * QB
        if bi == last:
            pv = ps_a.tile([DK1, QB], F32, tag="a", name="pv_last")
        else:
            pv = ps_pv.tile([DK1, QB], F32, tag="pv")
        cur_sc = []
        for kt in range(NKT):
            ps = ps_a.tile([128, QB], F32, tag="a")
            for u in range(QB // 512):
                nc.tensor.matmul(
                    ps[:, bass.ts(u, 512)],
                    kh_sb[ht][hr:hr + 64, bass.ts(kt, 128)],
                    qh_sb[ht][hr:hr + 64, q0 + u * 512:q0 + (u + 1) * 512],
                    start=True, stop=True,
                )
            sct = sctpool.tile([128, QB], BF16, tag="sct", name="sct")
            nc.scalar.activation(sct[:], ps[:], EXPF, scale=0.125)
            cur_sc.append(sct)
            if prev is not None:
                ppv, psc, ph, pqb = prev
                for u in range(QB // 512):
                    nc.tensor.matmul(
                        ppv[:, bass.ts(u, 512)],
                        v_sb[kt][:, ph * DK1:(ph + 1) * DK1],
                        psc[kt][:, bass.ts(u, 512)],
                        start=(kt == 0), stop=(kt == NKT - 1),
                    )
            if bi == last and kt >= 1:
                # self-interleave: the last block folds its own PV in with
                # a one-slot lag so the drain after the loop is only kt=15.
                for u in range(QB // 512):
                    nc.tensor.matmul(
                        pv[:, bass.ts(u, 512)],
                        v_sb[kt - 1][:, h * DK1:(h + 1) * DK1],
                        cur_sc[kt - 1][:, bass.ts(u, 512)],
                        start=(kt - 1 == 0), stop=False,
                    )
        if prev is not None:
            norm_and_out(prev[0], prev[2], prev[3])
        prev = (pv, cur_sc, h, qb)

    # drain: the last block only needs kt=15
    ppv, psc, ph, pqb = prev
    for u in range(QB // 512):
        nc.tensor.matmul(
            ppv[:, bass.ts(u, 512)],
            v_sb[NKT - 1][:, ph * DK1:(ph + 1) * DK1],
            psc[NKT - 1][:, bass.ts(u, 512)],
            start=False, stop=True,
        )
    norm_and_out(ppv, ph, pqb)
    for c in range(NCH):
        out_proj(c)


def _prep_inputs(q, k, v, Wq, bq, Wk, bk, Wv, bv, Wo, bo):
    """Build the per-core input maps (host-side sharding)."""
    in_maps = []
    for c in range(N_CORES):
        b, hg = c // 4, c % 4
        fsl = slice(FEAT * hg, FEAT * (hg + 1))
        osl = slice(OCOL * hg, OCOL * (hg + 1))
        wv_aug = np.zeros((D, VW), np.float32)
        bv_aug = np.zeros((VW,), np.float32)
        for h in range(HPC):
            rows = slice(FEAT * hg + DK * h, FEAT * hg + DK * (h + 1))
            wv_aug[:, h * DK1:h * DK1 + DK] = Wv[rows, :].T
            bv_aug[h * DK1:h * DK1 + DK] = bv[rows]
            bv_aug[h * DK1 + DK] = 1.0
        in_maps.append({
            "xq": np.ascontiguousarray(q[b].T).astype(NPBF16),
            "xk": np.ascontiguousarray(k[b].T).astype(NPBF16),
            "xv": np.ascontiguousarray(v[b].T).astype(NPBF16),
            "wq": np.ascontiguousarray(Wq[fsl].T).astype(NPBF16),
            "wk": np.ascontiguousarray(Wk[fsl].T).astype(NPBF16),
            "wv": wv_aug.astype(NPBF16),
            "wo": np.ascontiguousarray(Wo[osl].T).astype(NPBF16),
            "bq": np.ascontiguousarray(
                bq[fsl].reshape(2, 128).T).astype(np.float32),
            "bk": np.ascontiguousarray(
                bk[fsl].reshape(2, 128).T).astype(np.float32),
            "bv": bv_aug.reshape(1, VW).astype(NPBF16),
            "bo": np.ascontiguousarray(
                bo[osl].reshape(1, OCOL)).astype(NPBF16),
        })
    return in_maps


def run_sharded(in_maps, trace=False):
    nc = _build_program()
    res = run_bass_kernel_spmd(nc, in_maps, list(range(N_CORES)), trace=trace)
    full = np.empty((B, S, D), np.float32)
    for c in range(N_CORES):
        b, hg = c // 4, c % 4
        full[b, :, OCOL * hg:OCOL * (hg + 1)] = (
            res.results[c]["out"].astype(np.float32))
    return full, res


def kernel(q, k, v, Wq, bq, Wk, bk, Wv, bv, Wo, bo):
    args = [np.asarray(x, np.float32) for x in
            (q, k, v, Wq, bq, Wk, bk, Wv, bv, Wo, bo)]
    in_maps = _prep_inputs(*args)
    full, _ = run_sharded(in_maps)
    return full


# revision 15
# speedup vs baseline: 1.0790x; 1.0009x over previous
"""Multi-head attention (B=2, S=2048, D=1024, H=16) on 8 Trainium2 cores.

Sharding: core c handles batch c//4 and head-group c%4 (4 heads x dk 64).
Q/K/V projection weights are column-split by head group on the host; the
output projection is split by OUTPUT column: core c computes all 2048
tokens x its 256 output columns, so each core consumes the full gathered
concat but no final collective or dynamic slice is needed.

Attention runs in 4 chunks of 512 query tokens x 4 heads.  Scores stay in
[k, q] orientation; the PV product streams exp-scores against a stationary
V slice plus a ones column, giving [dk+1, q] with the softmax denominator
in row dk.  Normalization: DVE fast-reciprocal on the denominator row ->
gpsimd partition-broadcast -> fused multiply.  Scores of block i+1
interleave with the PV matmuls of block i so PE and ACT stay busy.

As soon as a chunk's 4 heads are normalized, an AllGather ships the
[256, 512] per-head outputs inside each 4-core batch group; the output
projection for that chunk is interleaved into a later chunk's attention
stream so the collective latency is hidden.  Only the last chunk's
AllGather + projection are exposed (~25us tail).
"""

import numpy as np
import ml_dtypes

import concourse.bass as bass
import concourse.tile as tile
from concourse import bacc, mybir
from concourse.bass_utils import run_bass_kernel_spmd

BF16 = mybir.dt.bfloat16
F32 = mybir.dt.float32
NPBF16 = ml_dtypes.bfloat16

B, S, D, H = 2, 2048, 1024, 16
DK = 64
DK1 = DK + 1
N_CORES = 8
HPC = 4               # heads per core
FEAT = HPC * DK       # 256 projected features per core
VW = HPC * DK1        # 260: v with a ones column per head
OCOL = 256            # output columns per core
TOKC = 1024           # token chunk for projections
QCH = 512             # q chunk for attention (= AllGather granularity)
NCH = S // QCH        # 4 chunks
NKT = S // 128        # 16 k tiles
NKC = D // 128        # 8 contraction chunks

_CACHE = {}


def _build_program():
    if "nc" in _CACHE:
        return _CACHE["nc"]

    nc = bacc.Bacc("TRN2", target_bir_lowering=False, debug=False,
                   num_devices=N_CORES)

    xq = nc.declare_dram_parameter("xq", [D, S], BF16, isOutput=False)
    xk = nc.declare_dram_parameter("xk", [D, S], BF16, isOutput=False)
    xv = nc.declare_dram_parameter("xv", [D, S], BF16, isOutput=False)
    wq = nc.declare_dram_parameter("wq", [D, FEAT], BF16, isOutput=False)
    wk = nc.declare_dram_parameter("wk", [D, FEAT], BF16, isOutput=False)
    wv = nc.declare_dram_parameter("wv", [D, VW], BF16, isOutput=False)
    wo = nc.declare_dram_parameter("wo", [D, OCOL], BF16, isOutput=False)
    bq = nc.declare_dram_parameter("bq", [128, 2], F32, isOutput=False)
    bk = nc.declare_dram_parameter("bk", [128, 2], F32, isOutput=False)
    bv = nc.declare_dram_parameter("bv", [1, VW], BF16, isOutput=False)
    bo = nc.declare_dram_parameter("bo", [1, OCOL], BF16, isOutput=False)
    out = nc.declare_dram_parameter("out", [S, OCOL], BF16, isOutput=True)
    dbg = {}

    with tile.TileContext(nc) as tc:
        with (
            tc.tile_pool(name="w", bufs=1) as wpool,
            tc.tile_pool(name="x", bufs=26) as xpool,
            tc.tile_pool(name="qk", bufs=1) as qkpool,
            tc.tile_pool(name="vp", bufs=1) as vpool,
            tc.tile_pool(name="sct", bufs=18) as sctpool,
            tc.tile_pool(name="nm", bufs=2) as nmpool,
            tc.tile_pool(name="cat", bufs=16) as catpool,
            tc.tile_pool(name="fo", bufs=3) as fopool,
            tc.tile_pool(name="ps_a", bufs=3, space="PSUM") as ps_a,
            tc.tile_pool(name="ps_pv", bufs=1, space="PSUM") as ps_pv,
            tc.tile_pool(name="dram", bufs=1, space="DRAM") as dram,
        ):
            _emit(nc, wpool, xpool, qkpool, vpool, sctpool, nmpool,
                  catpool, fopool, ps_a, ps_pv, dram,
                  xq, xk, xv, wq, wk, wv, wo, bq, bk, bv, bo, out, dbg)

    nc.compile()
    _CACHE["nc"] = nc
    return nc


def _emit(nc, wpool, xpool, qkpool, vpool, sctpool, nmpool, catpool,
          fopool, ps_a, ps_pv, dram,
          xq, xk, xv, wq, wk, wv, wo, bq, bk, bv, bo, out, dbg={}):
    MUL = mybir.AluOpType.mult
    EXPF = mybir.ActivationFunctionType.Exp
    IDF = mybir.ActivationFunctionType.Identity

    ones1 = wpool.tile([1, 128], BF16, tag="ones")
    nc.vector.memset(ones1[:], 1.0)

    # DMA issue engines for bulk input loads (round-robin: the Sync engine
    # alone issues descriptors at ~600ns each, which gates phase 1).  Only
    # SP and Activation are hardware-DGE engines; gpsimd DMA goes through
    # the software-DGE ring and corrupts data in this flow.
    dmae = [nc.sync, nc.scalar]
    NE = len(dmae)

    wk_sb = []
    for kc in range(NKC):
        t = wpool.tile([128, FEAT], BF16, tag=f"wk{kc}")
        dmae[kc % NE].dma_start(t[:], wk[bass.ts(kc, 128), :])
        wk_sb.append(t)
    bk_sb = wpool.tile([128, 2], F32, tag="bk")
    nc.sync.dma_start(bk_sb[:], bk[:])

    qh_sb = [qkpool.tile([128, S], BF16, tag=f"qh{m}", name=f"qh{m}")
             for m in range(2)]
    kh_sb = [qkpool.tile([128, S], BF16, tag=f"kh{m}", name=f"kh{m}")
             for m in range(2)]
    v_sb = [vpool.tile([128, VW], BF16, tag=f"v{j}", name=f"v{j}")
            for j in range(NKT)]

    def load_x(src, t0):
        tiles = []
        for kc in range(NKC):
            t = xpool.tile([128, TOKC], BF16, tag="xt")
            dmae[kc % NE].dma_start(t[:], src[bass.ts(kc, 128),
                                              bass.ts(t0, TOKC)])
            tiles.append(t)
        return tiles

    def qk_group(w_sb, x_t, b_sb, dst, t0, m):
        ps = ps_a.tile([128, TOKC], F32, tag="a")
        for kc in range(NKC):
            for u in range(TOKC // 512):
                nc.tensor.matmul(
                    ps[:, bass.ts(u, 512)],
                    w_sb[kc][:, bass.ts(m, 128)],
                    x_t[kc][:, bass.ts(u, 512)],
                    start=(kc == 0), stop=(kc == NKC - 1),
                )
        nc.vector.tensor_scalar_add(dst[m][:, bass.ts(t0, TOKC)], ps[:],
                                    b_sb[:, m:m + 1])

    # ---- K projection (scores need the full kh) ------------------
    xk_ts = [load_x(xk, t0) for t0 in range(S // TOKC)]
    for t0 in range(S // TOKC):
        for m in range(2):
            qk_group(wk_sb, xk_ts[t0], bk_sb, kh_sb, t0, m)

    wq_sb = []
    for kc in range(NKC):
        t = wpool.tile([128, FEAT], BF16, tag=f"wq{kc}")
        dmae[kc % NE].dma_start(t[:], wq[bass.ts(kc, 128), :])
        wq_sb.append(t)
    bq_sb = wpool.tile([128, 2], F32, tag="bq")
    nc.sync.dma_start(bq_sb[:], bq[:])
    for t0 in range(S // TOKC):
        xq_t = load_x(xq, t0)
        for m in range(2):
            qk_group(wq_sb, xq_t, bq_sb, qh_sb, t0, m)

    # ---- V weights + inputs --------------------------------------
    wv_sb = []
    for kc in range(NKC):
        t = wpool.tile([128, VW], BF16, tag=f"wv{kc}")
        dmae[kc % NE].dma_start(t[:], wv[bass.ts(kc, 128), :])
        wv_sb.append(t)
    bv_sb = wpool.tile([1, VW], BF16, tag="bv")
    nc.sync.dma_start(bv_sb[:], bv[:])
    xv_ts = [load_x(xv, t0) for t0 in range(S // TOKC)]

    def v_group(t0, j):
        ps = ps_a.tile([128, VW], F32, tag="a")
        for kc in range(NKC):
            nc.tensor.matmul(
                ps[:], xv_ts[t0][kc][:, bass.ts(j, 128)], wv_sb[kc][:],
                start=(kc == 0), stop=False,
            )
        nc.tensor.matmul(ps[:], ones1[:], bv_sb[:], start=False, stop=True)
        nc.vector.tensor_copy(v_sb[t0 * (TOKC // 128) + j][:], ps[:])

    for t0 in range(S // TOKC):
        for j in range(TOKC // 128):
            v_group(t0, j)

    # wo + bo requested now: the 0.5 MB load drains during attention.
    wo_sb = []
    for kc in range(NKC):
        t = wpool.tile([128, OCOL], BF16, tag=f"wo{kc}")
        dmae[kc % NE].dma_start(t[:], wo[bass.ts(kc, 128), :])
        wo_sb.append(t)
    bo_sb = wpool.tile([1, OCOL], BF16, tag="bo")
    nc.sync.dma_start(bo_sb[:], bo[:])

    # ---- phase 2/3: attention + chunked AllGather + out proj -----
    # Attention runs in 2 q-blocks of 1024 x 4 heads (the baseline shape,
    # which paces best under the power throttle).  AllGathers fire per
    # 512-token chunk as soon as its heads are normalized; chunk 3 ships
    # in two half-gathers (heads 01 early, heads 23 at the end).  The
    # column-split output projections all run after attention, where the
    # ACT engine is quiet and the PE runs unthrottled; their collectives
    # are complete by then, so there is no dead zone.
    QB = 2 * QCH          # 1024-token attention block
    NQB = S // QB         # 2 blocks
    ag_in = [dram.tile([FEAT, QCH], BF16, tag=f"agi{c}", name=f"agi{c}")
             for c in range(NCH)]
    ag_out = [dram.tile([4 * FEAT, QCH], BF16, tag=f"ago{c}", name=f"ago{c}")
              for c in range(NCH)]

    def emit_ag(eng, ins_ap, outs_ap):
        eng.collective_compute(
            "AllGather", mybir.AluOpType.bypass,
            replica_groups=[[0, 1, 2, 3], [4, 5, 6, 7]],
            ins=[ins_ap.opt()],
            outs=[outs_ap.opt()],
        )

    def norm_and_out(pv, h, qb):
        pvs = nmpool.tile([DK1, QB], F32, tag="pvs")
        nc.vector.tensor_copy(pvs[:], pv[:])
        drow = nmpool.tile([1, QB], BF16, tag="drow")
        nc.vector.tensor_copy(drow[:], pvs[DK:DK1, :])
        # Broadcast the denominator row via a rank-1 PE matmul instead of
        # gpsimd partition_broadcast, keeping the gpsimd queue free for the
        # AllGathers (its queue blocks while a collective is in flight).
        psb = ps_a.tile([DK, QB], F32, tag="a", name="psb")
        for u in range(QB // 512):
            nc.tensor.matmul(psb[:, bass.ts(u, 512)], ones1[0:1, 0:DK],
                             drow[:, bass.ts(u, 512)], start=True, stop=True)
        dbs = nmpool.tile([DK, QB], F32, tag="db")
        nc.vector.tensor_copy(dbs[:], psb[:])
        rb = nmpool.tile([DK, QB], F32, tag="rb")
        nc.vector.reciprocal_approx_fast(rb[:], dbs[:])
        onrm = nmpool.tile([DK, QB], BF16, tag="onrm")
        nc.vector.scalar_tensor_tensor(onrm[:], pvs[0:DK, :], 1.0, rb[:],
                                       MUL, MUL)
        for u in range(QB // QCH):
            c = qb * (QB // QCH) + u
            nc.sync.dma_start(ag_in[c][h * DK:(h + 1) * DK, :],
                              onrm[:, bass.ts(u, QCH)])
        if h == HPC - 1:
            for u in range(QB // QCH):
                c = qb * (QB // QCH) + u
                emit_ag(nc.gpsimd, ag_in[c][:], ag_out[c][:])

    def out_proj(c):
        cat = []
        for kc in range(NKC):
            t = catpool.tile([128, QCH], BF16, tag="cat")
            dmae[kc % NE].dma_start(t[:], ag_out[c][bass.ts(kc, 128), :])
            cat.append(t)
        for qt in range(QCH // 128):
            ps = ps_a.tile([128, OCOL], F32, tag="a", name="po")
            nc.tensor.matmul(ps[:], ones1[:, 0:128], bo_sb[:],
                             start=True, stop=False)
            for kc in range(NKC):
                nc.tensor.matmul(
                    ps[:],
                    cat[kc][:, bass.ts(qt, 128)],
                    wo_sb[kc][:],
                    start=False, stop=(kc == NKC - 1),
                )
            fo = fopool.tile([128, OCOL], BF16, tag="fo")
            nc.scalar.activation(fo[:], ps[:], IDF)
            dmae[qt % NE].dma_start(
                out[bass.ts(c * (QCH // 128) + qt, 128), :], fo[:])

    blocks = [(qb, h) for qb in range(NQB) for h in range(HPC)]
    last = len(blocks) - 1
    prev = None
    for bi, (qb, h) in enumerate(blocks):
        ht, hr = h // 2, (h % 2) * 64
        q0 = qb * QB
        if bi == last:
            pv = ps_a.tile([DK1, QB], F32, tag="a", name="pv_last")
        else:
            pv = ps_pv.tile([DK1, QB], F32, tag="pv")
        cur_sc = []
        for kt in range(NKT):
            ps = ps_a.tile([128, QB], F32, tag="a")
            for u in range(QB // 512):
                nc.tensor.matmul(
                    ps[:, bass.ts(u, 512)],
                    kh_sb[ht][hr:hr + 64, bass.ts(kt, 128)],
                    qh_sb[ht][hr:hr + 64, q0 + u * 512:q0 + (u + 1) * 512],
                    start=True, stop=True,
                )
            sct = sctpool.tile([128, QB], BF16, tag="sct", name="sct")
            nc.scalar.activation(sct[:], ps[:], EXPF, scale=0.125)
            cur_sc.append(sct)
            if prev is not None:
                ppv, psc, ph, pqb = prev
                for u in range(QB // 512):
                    nc.tensor.matmul(
                        ppv[:, bass.ts(u, 512)],
                        v_sb[kt][:, ph * DK1:(ph + 1) * DK1],
                        psc[kt][:, bass.ts(u, 512)],
                        start=(kt == 0), stop=(kt == NKT - 1),
                    )
            if bi == last and kt >= 1:
                # self-interleave: the last block folds its own PV in with
                # a one-slot lag so the drain after the loop is only kt=15.
                for u in range(QB // 512):
                    nc.tensor.matmul(
                        pv[:, bass.ts(u, 512)],
                        v_sb[kt - 1][:, h * DK1:(h + 1) * DK1],
                        cur_sc[kt - 1][:, bass.ts(u, 512)],
                        start=(kt - 1 == 0), stop=False,
                    )
        if prev is not None:
            norm_and_out(prev[0], prev[2], prev[3])
        prev = (pv, cur_sc, h, qb)

    # drain: the last block only needs kt=15
    ppv, psc, ph, pqb = prev
    for u in range(QB // 512):
        nc.tensor.matmul(
            ppv[:, bass.ts(u, 512)],
            v_sb[NKT - 1][:, ph * DK1:(ph + 1) * DK1],
            psc[NKT - 1][:, bass.ts(u, 512)],
            start=False, stop=True,
        )
    norm_and_out(ppv, ph, pqb)
    for c in range(NCH):
        out_proj(c)


def _prep_inputs(q, k, v, Wq, bq, Wk, bk, Wv, bv, Wo, bo):
    """Build the per-core input maps (host-side sharding)."""
    in_maps = []
    for c in range(N_CORES):
        b, hg = c // 4, c % 4
        fsl = slice(FEAT * hg, FEAT * (hg + 1))
        osl = slice(OCOL * hg, OCOL * (hg + 1))
        wv_aug = np.zeros((D, VW), np.float32)
        bv_aug = np.zeros((VW,), np.float32)
        for h in range(HPC):
            rows = slice(FEAT * hg + DK * h, FEAT * hg + DK * (h + 1))
            wv_aug[:, h * DK1:h * DK1 + DK] = Wv[rows, :].T
            bv_aug[h * DK1:h * DK1 + DK] = bv[rows]
            bv_aug[h * DK1 + DK] = 1.0
        in_maps.append({
            "xq": np.ascontiguousarray(q[b].T).astype(NPBF16),
            "xk": np.ascontiguousarray(k[b].T).astype(NPBF16),
            "xv": np.ascontiguousarray(v[b].T).astype(NPBF16),
            "wq": np.ascontiguousarray(Wq[fsl].T).astype(NPBF16),
            "wk": np.ascontiguousarray(Wk[fsl].T).astype(NPBF16),
            "wv": wv_aug.astype(NPBF16),
            "wo": np.ascontiguousarray(Wo[osl].T).astype(NPBF16),
            "bq": np.ascontiguousarray(
                bq[fsl].reshape(2, 128).T).astype(np.float32),
            "bk": np.ascontiguousarray(
                bk[fsl].reshape(2, 128).T).astype(np.float32),
            "bv": bv_aug.reshape(1, VW).astype(NPBF16),
            "bo": np.ascontiguousarray(
                bo[osl].reshape(1, OCOL)).astype(NPBF16),
        })
    return in_maps


def run_sharded(in_maps, trace=False):
    nc = _build_program()
    res = run_bass_kernel_spmd(nc, in_maps, list(range(N_CORES)), trace=trace)
    full = np.empty((B, S, D), np.float32)
    for c in range(N_CORES):
        b, hg = c // 4, c % 4
        full[b, :, OCOL * hg:OCOL * (hg + 1)] = (
            res.results[c]["out"].astype(np.float32))
    return full, res


def kernel(q, k, v, Wq, bq, Wk, bk, Wv, bv, Wo, bo):
    args = [np.asarray(x, np.float32) for x in
            (q, k, v, Wq, bq, Wk, bk, Wv, bv, Wo, bo)]
    in_maps = _prep_inputs(*args)
    full, _ = run_sharded(in_maps)
    return full
